# revision 41
# baseline (speedup 1.0000x reference)
"""ErrorMemoryBank.retrieve kernel for 8 TRN2 NeuronCores.

Strategy (bank sharded on the capacity axis, 12500 keys per core):
  host prep : fold temporal decay and key L2-norm into a per-key scale,
              pre-scale the keys once (shipped as bf16), L2-normalize +
              transpose the query.
  device    : per core, stream the key shard through the PE array
              (transpose + bf16 matmul against the 128x256 query blocks)
              and reduce the 256x12800 similarity matrix to per-segment
              maxima (segments of 32 keys) on the vector engine.  Only
              [256, 400] f32 seg-maxima leave each core.
  host post : pick the top-M segments per query from the 8*400 global
              seg-maxima, recompute exact fp32 similarities for those few
              candidates, take the global top-k (lowest-index
              tie-breaking, matching jax.lax.top_k), gather values.

The union of the top-k segments (ranked by segment max) always contains
the top-k elements, so M >= top_k plus a generous margin + widening
fallback makes host selection exact up to fp rounding.
"""

import os
import sys

import numpy as np

for _p in ("/opt/trn_rl_repo",):
    if _p not in sys.path:
        sys.path.insert(0, _p)

N_CORES = 8
B, D = 256, 128
CAP = 100000
C = CAP // N_CORES   # 12500 real keys per core
W = 512              # matmul chunk width (columns)
R = 4                # keys packed per SBUF partition (1KB runs)
TP = W // R          # 128 partitions per raw keys tile
NT = R               # transpose strips per chunk
NCH = 25             # chunks per core
CD = NCH * W         # 12800 device keys per core (padded with zeros)
SEG = 32             # keys per segment (seg-max granularity)
SPC = W // SEG       # 16 segments per chunk
G = NCH * SPC        # 400 segments per core
NB = B // 128        # 2 query partition blocks
SPLIT_K = 15         # chunk after which the first output half is drained

DECAY_FACTOR = 0.995
MIN_SIMILARITY = 0.0
NORM_EPS = 1e-12
MARGIN = 1e-2        # device (bf16) seg-max error allowance for the fallback

LAST_EXEC_TIME_NS = None
LAST_RESULTS = None

_NC_CACHE = {}

# The per-chunk DMA packs R consecutive keys into each SBUF partition
# (2KB contiguous runs), so sims column c of chunk k corresponds to
# per-core key k*W + R*(c % TP) + c // TP.  Each core is padded from C
# to CD keys; COL2KEY maps a global sims column to its global key id,
# with -1 marking pad columns.
_gc = np.arange(N_CORES * CD, dtype=np.int64)
_core = _gc // CD
_lc = _gc % CD
_cc = _lc % W
_lkey = (_lc - _cc) + R * (_cc % TP) + _cc // TP
COL2KEY = np.where(_lkey < C, _core * C + _lkey, -1)
del _gc, _core, _lc, _cc, _lkey


def _build_bass():
    import concourse.mybir as mybir
    from concourse import bacc
    from concourse.tile import TileContext

    f32 = mybir.dt.float32
    bf16 = mybir.dt.bfloat16

    nc = bacc.Bacc(trn_type="TRN2")
    # qc packs the transposed query [D, B] and a DxD identity (for the PE
    # transposes) into one input so a single early DMA covers both.
    qc_d = nc.declare_dram_parameter("qc", [D, B + TP], bf16, isOutput=False)
    keys_d = nc.declare_dram_parameter("keys", [CD, D], bf16, isOutput=False)
    segmax_d = nc.declare_dram_parameter("segmax", [B, G], f32, isOutput=True)

    with TileContext(nc) as tc:
        with tc.tile_pool(name="const", bufs=1) as constp, \
             tc.tile_pool(name="kraw", bufs=4) as kraw, \
             tc.tile_pool(name="knt", bufs=3) as kntp, \
             tc.tile_pool(name="segout", bufs=1) as segp, \
             tc.tile_pool(name="tpsum", bufs=2, space="PSUM") as tpsum, \
             tc.tile_pool(name="mpsum", bufs=3, space="PSUM") as mpsum:

            qc = constp.tile([D, B + TP], bf16, tag="qc")
            nc.sync.dma_start(out=qc[:], in_=qc_d[:])
            qnT = qc[:, :B]
            ident = qc[:, B:]

            seg = segp.tile([D, NB, G], f32, tag="seg")

            # fused per-chunk pipeline: DMA (R keys packed per partition,
            # 1KB contiguous bf16 runs) -> 4 PE transposes -> ACT copy ->
            # 2 matmuls -> 1 segmented reduce.  Column c of chunk k holds
            # key k*W + R*(c % TP) + c // TP (host undoes the packing).
            dma_engines = [nc.sync, nc.scalar]
            for k in range(NCH):
                kt = kraw.tile([TP, R * D], bf16, tag="kt")
                dma_engines[k % len(dma_engines)].dma_start(
                    out=kt[:],
                    in_=keys_d[k * W:(k + 1) * W, :].rearrange(
                        "(p j) d -> p (j d)", j=R),
                )
                ps = tpsum.tile([D, 512], bf16, tag="tp")
                for j in range(NT):
                    nc.tensor.transpose(
                        out=ps[:, j * TP:(j + 1) * TP],
                        in_=kt[:, j * D:(j + 1) * D],
                        identity=ident,
                    )
                knt = kntp.tile([D, W], bf16, tag="knt")
                nc.scalar.copy(out=knt[:], in_=ps[:])

                mp = mpsum.tile([D, NB, 512], f32, tag="mp")
                for pb in range(NB):
                    nc.tensor.matmul(
                        out=mp[:, pb, :],
                        lhsT=qnT[:, pb * 128:(pb + 1) * 128],
                        rhs=knt[:],
                        start=True, stop=True,
                    )
                red_in = mp[:].rearrange("p a (g s) -> p a g s", s=SEG)
                nc.vector.reduce_max(
                    out=seg[:, :, k * SPC:(k + 1) * SPC],
                    in_=red_in,
                    axis=mybir.AxisListType.X,
                )
                if k == SPLIT_K:
                    # drain the finished half of the output early so the
                    # kernel tail only pays for the remainder.
                    g0 = (SPLIT_K + 1) * SPC
                    for pb in range(NB):
                        nc.sync.dma_start(
                            out=segmax_d[pb * 128:(pb + 1) * 128, :g0],
                            in_=seg[:, pb, :g0],
                        )

            g0 = (SPLIT_K + 1) * SPC
            for pb in range(NB):
                nc.sync.dma_start(
                    out=segmax_d[pb * 128:(pb + 1) * 128, g0:],
                    in_=seg[:, pb, g0:],
                )

    nc.finalize()
    return nc


def _get_nc():
    if "nc" not in _NC_CACHE:
        _NC_CACHE["nc"] = _build_bass()
    return _NC_CACHE["nc"]


def _install_trace_hook():
    """Make trace=True work under axon: provide the antenv.axon_hooks
    shim the boot script expects and register the ctypes NTFF hook."""
    import types

    try:
        from antenv.axon_hooks import get_axon_ntff_profile_hook  # noqa: F401
        return True
    except ImportError:
        pass
    try:
        import antenv
        from trn_agent_boot.trn_boot import _ntff_profile_via_ctypes

        m = types.ModuleType("antenv.axon_hooks")
        _state = {"hook": None}
        m.set_axon_ntff_profile_hook = lambda h: _state.__setitem__("hook", h)
        m.get_axon_ntff_profile_hook = lambda: _state["hook"]
        sys.modules["antenv.axon_hooks"] = m
        antenv.axon_hooks = m
        hook = _ntff_profile_via_ctypes("/opt/axon/libaxon_pjrt.so")
        if hook is None:
            return False
        m.set_axon_ntff_profile_hook(hook)

        # artifact upload needs bucket creds the container lacks; keep
        # the profile local instead.
        from concourse import bass_utils as bu
        bu.upload_artifacts = lambda tmpdir: f"local://{tmpdir}"
        return True
    except Exception as e:  # pragma: no cover
        print("trace hook install failed:", e)
        return False


def _host_topk(A, qn, keys_scaled, top_k, M):
    """Select top-k elements per query given device seg-maxima A [B, NSEG]."""
    nseg = A.shape[1]
    M = min(M, nseg)
    idx = np.argpartition(-A, M - 1, axis=1)[:, :M]           # [B, M] seg ids
    if M < nseg:
        thr = -np.partition(-A, M, axis=1)[:, M]              # (M+1)-th seg max
    else:
        thr = np.full(A.shape[0], -np.inf, dtype=A.dtype)
    cols = (idx[:, :, None] * SEG
            + np.arange(SEG, dtype=np.int64)).reshape(A.shape[0], M * SEG)
    cand = np.sort(COL2KEY[cols], axis=1)     # -1 pad entries sort first
    valid = cand >= 0
    kc = keys_scaled[np.where(valid, cand, 0)]                 # [B, M*SEG, D]
    sims = np.matmul(kc, qn[:, :, None], dtype=np.float32)[:, :, 0]
    sims[~valid] = -np.inf
    order = np.argsort(-sims, axis=1, kind="stable")[:, :top_k]
    top_idx = np.take_along_axis(cand, order, axis=1)
    top_sims = np.take_along_axis(sims, order, axis=1)
    safe = top_sims[:, -1] >= thr + MARGIN
    return top_idx, top_sims, safe


def kernel(**inputs):
    global LAST_EXEC_TIME_NS, LAST_RESULTS

    query = np.ascontiguousarray(np.asarray(inputs["query"], dtype=np.float32))
    keys = np.ascontiguousarray(np.asarray(inputs["keys"], dtype=np.float32))
    values = np.asarray(inputs["values"], dtype=np.float32)
    ts = np.asarray(inputs["timestamps"])
    gs = int(np.asarray(inputs["global_step"]))
    top_k = int(np.asarray(inputs["top_k"]))

    # --- host prep -------------------------------------------------------
    qn = query / np.maximum(
        np.linalg.norm(query, axis=-1, keepdims=True), NORM_EPS)
    qn = qn.astype(np.float32)
    qnT = np.ascontiguousarray(qn.T)

    age = (np.int64(gs) - ts.astype(np.int64)).astype(np.float32)
    decay = np.power(np.float32(DECAY_FACTOR), age).astype(np.float32)
    knorm = np.sqrt(np.einsum("cd,cd->c", keys, keys)).astype(np.float32)
    scale = decay / np.maximum(knorm, NORM_EPS)
    keys_scaled = keys * scale[:, None]

    import ml_dtypes
    qc = np.zeros((D, B + TP), dtype=ml_dtypes.bfloat16)
    qc[:, :B] = qnT.astype(ml_dtypes.bfloat16)
    qc[:, B:] = np.eye(D, dtype=ml_dtypes.bfloat16)
    keys_bf = np.zeros((N_CORES * CD, D), dtype=ml_dtypes.bfloat16)
    keys_bf.reshape(N_CORES, CD, D)[:, :C, :] = \
        keys_scaled.astype(ml_dtypes.bfloat16).reshape(N_CORES, C, D)
    in_maps = [
        {"qc": qc, "keys": keys_bf[i * CD:(i + 1) * CD]}
        for i in range(N_CORES)
    ]

    # --- device ----------------------------------------------------------
    from concourse.bass_utils import run_bass_kernel_spmd

    nc = _get_nc()
    trace = os.environ.get("BASS_KERNEL_TRACE", "0") == "1"
    kw = {}
    if trace and _install_trace_hook():
        kw = {"trace": True,
              "trace_cores": list(range(N_CORES))}
    try:
        res = run_bass_kernel_spmd(nc, in_maps, list(range(N_CORES)), **kw)
    except Exception:
        if not kw:
            raise
        res = run_bass_kernel_spmd(nc, in_maps, list(range(N_CORES)))
    LAST_EXEC_TIME_NS = res.exec_time_ns
    LAST_RESULTS = res

    A = np.concatenate(
        [res.results[i]["segmax"] for i in range(N_CORES)], axis=1)

    # --- host reduce ------------------------------------------------------
    M = max(12, top_k)
    while True:
        top_idx, top_sims, safe = _host_topk(A, qn, keys_scaled, top_k, M)
        if safe.all() or M >= A.shape[1]:
            break
        M = min(M * 4, A.shape[1])

    retrieved = values[top_idx]
    valid_mask = top_sims >= np.float32(MIN_SIMILARITY)
    return (retrieved.astype(np.float32),
            top_sims.astype(np.float32),
            valid_mask,
            top_idx.astype(np.int32))


# revision 42
# speedup vs baseline: 1.0202x; 1.0202x over previous
"""ErrorMemoryBank.retrieve kernel for 8 TRN2 NeuronCores.

Strategy (bank sharded on the capacity axis, 12500 keys per core):
  host prep : fold temporal decay and key L2-norm into a per-key scale,
              pre-scale the keys once (shipped as bf16), L2-normalize +
              transpose the query.
  device    : per core, stream the key shard through the PE array
              (transpose + bf16 matmul against the 128x256 query blocks)
              and reduce the 256x12800 similarity matrix to per-segment
              maxima (segments of 32 keys) on the vector engine.  Only
              [256, 400] f32 seg-maxima leave each core.
  host post : pick the top-M segments per query from the 8*400 global
              seg-maxima, recompute exact fp32 similarities for those few
              candidates, take the global top-k (lowest-index
              tie-breaking, matching jax.lax.top_k), gather values.

The union of the top-k segments (ranked by segment max) always contains
the top-k elements, so M >= top_k plus a generous margin + widening
fallback makes host selection exact up to fp rounding.
"""

import os
import sys

import numpy as np

for _p in ("/opt/trn_rl_repo",):
    if _p not in sys.path:
        sys.path.insert(0, _p)

N_CORES = 8
B, D = 256, 128
CAP = 100000
C = CAP // N_CORES   # 12500 real keys per core
W = 512              # matmul chunk width (columns)
R = 4                # keys packed per SBUF partition (1KB runs)
TP = W // R          # 128 partitions per raw keys tile
NT = R               # transpose strips per chunk
NCH = 25             # chunks per core
CD = NCH * W         # 12800 device keys per core (padded with zeros)
SEG = 32             # keys per segment (seg-max granularity)
SPC = W // SEG       # 16 segments per chunk
G = NCH * SPC        # 400 segments per core
NB = B // 128        # 2 query partition blocks
SPLIT_K = 15         # chunk after which the first output half is drained

DECAY_FACTOR = 0.995
MIN_SIMILARITY = 0.0
NORM_EPS = 1e-12
MARGIN = 1e-2        # device (bf16) seg-max error allowance for the fallback

LAST_EXEC_TIME_NS = None
LAST_RESULTS = None

_NC_CACHE = {}

# The per-chunk DMA packs R consecutive keys into each SBUF partition
# (2KB contiguous runs), so sims column c of chunk k corresponds to
# per-core key k*W + R*(c % TP) + c // TP.  Each core is padded from C
# to CD keys; COL2KEY maps a global sims column to its global key id,
# with -1 marking pad columns.
_gc = np.arange(N_CORES * CD, dtype=np.int64)
_core = _gc // CD
_lc = _gc % CD
_cc = _lc % W
_lkey = (_lc - _cc) + R * (_cc % TP) + _cc // TP
COL2KEY = np.where(_lkey < C, _core * C + _lkey, -1)
del _gc, _core, _lc, _cc, _lkey


def _build_bass():
    import concourse.mybir as mybir
    from concourse import bacc
    from concourse.tile import TileContext

    f32 = mybir.dt.float32
    bf16 = mybir.dt.bfloat16

    nc = bacc.Bacc(trn_type="TRN2")
    # qc packs the transposed query [D, B] and a DxD identity (for the PE
    # transposes) into one input so a single early DMA covers both.
    qc_d = nc.declare_dram_parameter("qc", [D, B + TP], bf16, isOutput=False)
    keys_d = nc.declare_dram_parameter("keys", [CD, D], bf16, isOutput=False)
    segmax_d = nc.declare_dram_parameter("segmax", [B, G], f32, isOutput=True)

    with TileContext(nc) as tc:
        with tc.tile_pool(name="const", bufs=1) as constp, \
             tc.tile_pool(name="kraw", bufs=4) as kraw, \
             tc.tile_pool(name="knt", bufs=3) as kntp, \
             tc.tile_pool(name="segout", bufs=1) as segp, \
             tc.tile_pool(name="tpsum", bufs=2, space="PSUM") as tpsum, \
             tc.tile_pool(name="mpsum", bufs=3, space="PSUM") as mpsum:

            qc = constp.tile([D, B + TP], bf16, tag="qc")
            nc.sync.dma_start(out=qc[:], in_=qc_d[:])
            qnT = qc[:, :B]
            ident = qc[:, B:]

            seg = segp.tile([D, NB, G], f32, tag="seg")

            # fused per-chunk pipeline: DMA (R keys packed per partition,
            # 1KB contiguous bf16 runs) -> 4 PE transposes -> ACT copy ->
            # 2 matmuls -> 1 segmented reduce.  Column c of chunk k holds
            # key k*W + R*(c % TP) + c // TP (host undoes the packing).
            dma_engines = [nc.sync]
            for k in range(NCH):
                kt = kraw.tile([TP, R * D], bf16, tag="kt")
                dma_engines[k % len(dma_engines)].dma_start(
                    out=kt[:],
                    in_=keys_d[k * W:(k + 1) * W, :].rearrange(
                        "(p j) d -> p (j d)", j=R),
                )
                ps = tpsum.tile([D, 512], bf16, tag="tp")
                for j in range(NT):
                    nc.tensor.transpose(
                        out=ps[:, j * TP:(j + 1) * TP],
                        in_=kt[:, j * D:(j + 1) * D],
                        identity=ident,
                    )
                knt = kntp.tile([D, W], bf16, tag="knt")
                nc.scalar.copy(out=knt[:], in_=ps[:])

                mp = mpsum.tile([D, NB, 512], f32, tag="mp")
                for pb in range(NB):
                    nc.tensor.matmul(
                        out=mp[:, pb, :],
                        lhsT=qnT[:, pb * 128:(pb + 1) * 128],
                        rhs=knt[:],
                        start=True, stop=True,
                    )
                red_in = mp[:].rearrange("p a (g s) -> p a g s", s=SEG)
                nc.vector.reduce_max(
                    out=seg[:, :, k * SPC:(k + 1) * SPC],
                    in_=red_in,
                    axis=mybir.AxisListType.X,
                )
                if k == SPLIT_K:
                    # drain the finished half of the output early so the
                    # kernel tail only pays for the remainder.
                    g0 = (SPLIT_K + 1) * SPC
                    for pb in range(NB):
                        nc.sync.dma_start(
                            out=segmax_d[pb * 128:(pb + 1) * 128, :g0],
                            in_=seg[:, pb, :g0],
                        )

            g0 = (SPLIT_K + 1) * SPC
            for pb in range(NB):
                nc.sync.dma_start(
                    out=segmax_d[pb * 128:(pb + 1) * 128, g0:],
                    in_=seg[:, pb, g0:],
                )

    nc.finalize()
    return nc


def _get_nc():
    if "nc" not in _NC_CACHE:
        _NC_CACHE["nc"] = _build_bass()
    return _NC_CACHE["nc"]


def _install_trace_hook():
    """Make trace=True work under axon: provide the antenv.axon_hooks
    shim the boot script expects and register the ctypes NTFF hook."""
    import types

    try:
        from antenv.axon_hooks import get_axon_ntff_profile_hook  # noqa: F401
        return True
    except ImportError:
        pass
    try:
        import antenv
        from trn_agent_boot.trn_boot import _ntff_profile_via_ctypes

        m = types.ModuleType("antenv.axon_hooks")
        _state = {"hook": None}
        m.set_axon_ntff_profile_hook = lambda h: _state.__setitem__("hook", h)
        m.get_axon_ntff_profile_hook = lambda: _state["hook"]
        sys.modules["antenv.axon_hooks"] = m
        antenv.axon_hooks = m
        hook = _ntff_profile_via_ctypes("/opt/axon/libaxon_pjrt.so")
        if hook is None:
            return False
        m.set_axon_ntff_profile_hook(hook)

        # artifact upload needs bucket creds the container lacks; keep
        # the profile local instead.
        from concourse import bass_utils as bu
        bu.upload_artifacts = lambda tmpdir: f"local://{tmpdir}"
        return True
    except Exception as e:  # pragma: no cover
        print("trace hook install failed:", e)
        return False


def _host_topk(A, qn, keys_scaled, top_k, M):
    """Select top-k elements per query given device seg-maxima A [B, NSEG]."""
    nseg = A.shape[1]
    M = min(M, nseg)
    idx = np.argpartition(-A, M - 1, axis=1)[:, :M]           # [B, M] seg ids
    if M < nseg:
        thr = -np.partition(-A, M, axis=1)[:, M]              # (M+1)-th seg max
    else:
        thr = np.full(A.shape[0], -np.inf, dtype=A.dtype)
    cols = (idx[:, :, None] * SEG
            + np.arange(SEG, dtype=np.int64)).reshape(A.shape[0], M * SEG)
    cand = np.sort(COL2KEY[cols], axis=1)     # -1 pad entries sort first
    valid = cand >= 0
    kc = keys_scaled[np.where(valid, cand, 0)]                 # [B, M*SEG, D]
    sims = np.matmul(kc, qn[:, :, None], dtype=np.float32)[:, :, 0]
    sims[~valid] = -np.inf
    order = np.argsort(-sims, axis=1, kind="stable")[:, :top_k]
    top_idx = np.take_along_axis(cand, order, axis=1)
    top_sims = np.take_along_axis(sims, order, axis=1)
    safe = top_sims[:, -1] >= thr + MARGIN
    return top_idx, top_sims, safe


def kernel(**inputs):
    global LAST_EXEC_TIME_NS, LAST_RESULTS

    query = np.ascontiguousarray(np.asarray(inputs["query"], dtype=np.float32))
    keys = np.ascontiguousarray(np.asarray(inputs["keys"], dtype=np.float32))
    values = np.asarray(inputs["values"], dtype=np.float32)
    ts = np.asarray(inputs["timestamps"])
    gs = int(np.asarray(inputs["global_step"]))
    top_k = int(np.asarray(inputs["top_k"]))

    # --- host prep -------------------------------------------------------
    qn = query / np.maximum(
        np.linalg.norm(query, axis=-1, keepdims=True), NORM_EPS)
    qn = qn.astype(np.float32)
    qnT = np.ascontiguousarray(qn.T)

    age = (np.int64(gs) - ts.astype(np.int64)).astype(np.float32)
    decay = np.power(np.float32(DECAY_FACTOR), age).astype(np.float32)
    knorm = np.sqrt(np.einsum("cd,cd->c", keys, keys)).astype(np.float32)
    scale = decay / np.maximum(knorm, NORM_EPS)
    keys_scaled = keys * scale[:, None]

    import ml_dtypes
    qc = np.zeros((D, B + TP), dtype=ml_dtypes.bfloat16)
    qc[:, :B] = qnT.astype(ml_dtypes.bfloat16)
    qc[:, B:] = np.eye(D, dtype=ml_dtypes.bfloat16)
    keys_bf = np.zeros((N_CORES * CD, D), dtype=ml_dtypes.bfloat16)
    keys_bf.reshape(N_CORES, CD, D)[:, :C, :] = \
        keys_scaled.astype(ml_dtypes.bfloat16).reshape(N_CORES, C, D)
    in_maps = [
        {"qc": qc, "keys": keys_bf[i * CD:(i + 1) * CD]}
        for i in range(N_CORES)
    ]

    # --- device ----------------------------------------------------------
    from concourse.bass_utils import run_bass_kernel_spmd

    nc = _get_nc()
    trace = os.environ.get("BASS_KERNEL_TRACE", "0") == "1"
    kw = {}
    if trace and _install_trace_hook():
        kw = {"trace": True,
              "trace_cores": list(range(N_CORES))}
    try:
        res = run_bass_kernel_spmd(nc, in_maps, list(range(N_CORES)), **kw)
    except Exception:
        if not kw:
            raise
        res = run_bass_kernel_spmd(nc, in_maps, list(range(N_CORES)))
    LAST_EXEC_TIME_NS = res.exec_time_ns
    LAST_RESULTS = res

    A = np.concatenate(
        [res.results[i]["segmax"] for i in range(N_CORES)], axis=1)

    # --- host reduce ------------------------------------------------------
    M = max(12, top_k)
    while True:
        top_idx, top_sims, safe = _host_topk(A, qn, keys_scaled, top_k, M)
        if safe.all() or M >= A.shape[1]:
            break
        M = min(M * 4, A.shape[1])

    retrieved = values[top_idx]
    valid_mask = top_sims >= np.float32(MIN_SIMILARITY)
    return (retrieved.astype(np.float32),
            top_sims.astype(np.float32),
            valid_mask,
            top_idx.astype(np.int32))


# revision 44
# speedup vs baseline: 1.0467x; 1.0260x over previous
"""ErrorMemoryBank.retrieve kernel for 8 TRN2 NeuronCores.

Strategy (bank sharded on the capacity axis, 12500 keys per core):
  host prep : fold temporal decay and key L2-norm into a per-key scale,
              pre-scale the keys once (shipped as bf16), L2-normalize +
              transpose the query.
  device    : per core, stream the key shard through the PE array
              (transpose + bf16 matmul against the 128x256 query blocks)
              and reduce the 256x12800 similarity matrix to per-segment
              maxima (segments of 32 keys) on the vector engine.  Only
              [256, 400] f32 seg-maxima leave each core.
  host post : pick the top-M segments per query from the 8*400 global
              seg-maxima, recompute exact fp32 similarities for those few
              candidates, take the global top-k (lowest-index
              tie-breaking, matching jax.lax.top_k), gather values.

The union of the top-k segments (ranked by segment max) always contains
the top-k elements, so M >= top_k plus a generous margin + widening
fallback makes host selection exact up to fp rounding.
"""

import os
import sys

import numpy as np

for _p in ("/opt/trn_rl_repo",):
    if _p not in sys.path:
        sys.path.insert(0, _p)

N_CORES = 8
B, D = 256, 128
CAP = 100000
C = CAP // N_CORES   # 12500 real keys per core
W = 512              # matmul chunk width (columns)
R = 4                # keys packed per SBUF partition (1KB runs)
TP = W // R          # 128 partitions per raw keys tile
NT = R               # transpose strips per chunk
NCH = 25             # chunks per core
CD = NCH * W         # 12800 device keys per core (padded with zeros)
SEG = 32             # keys per segment (seg-max granularity)
SPC = W // SEG       # 16 segments per chunk
G = NCH * SPC        # 400 segments per core
NB = B // 128        # 2 query partition blocks
SPLIT_K = 15         # chunk after which the first output half is drained

DECAY_FACTOR = 0.995
MIN_SIMILARITY = 0.0
NORM_EPS = 1e-12
MARGIN = 1e-2        # device (bf16) seg-max error allowance for the fallback

LAST_EXEC_TIME_NS = None
LAST_RESULTS = None

_NC_CACHE = {}

# The per-chunk DMA packs R consecutive keys into each SBUF partition
# (2KB contiguous runs), so sims column c of chunk k corresponds to
# per-core key k*W + R*(c % TP) + c // TP.  Each core is padded from C
# to CD keys; COL2KEY maps a global sims column to its global key id,
# with -1 marking pad columns.
_gc = np.arange(N_CORES * CD, dtype=np.int64)
_core = _gc // CD
_lc = _gc % CD
_cc = _lc % W
_lkey = (_lc - _cc) + R * (_cc % TP) + _cc // TP
COL2KEY = np.where(_lkey < C, _core * C + _lkey, -1)
del _gc, _core, _lc, _cc, _lkey


def _build_bass():
    import concourse.mybir as mybir
    from concourse import bacc
    from concourse.tile import TileContext

    f32 = mybir.dt.float32
    bf16 = mybir.dt.bfloat16

    nc = bacc.Bacc(trn_type="TRN2")
    # qc packs the transposed query [D, B] and a DxD identity (for the PE
    # transposes) into one input so a single early DMA covers both.
    qc_d = nc.declare_dram_parameter("qc", [D, B + TP], bf16, isOutput=False)
    keys_d = nc.declare_dram_parameter("keys", [CD, D], bf16, isOutput=False)
    segmax_d = nc.declare_dram_parameter("segmax", [B, G], f32, isOutput=True)

    with TileContext(nc) as tc:
        with tc.tile_pool(name="const", bufs=1) as constp, \
             tc.tile_pool(name="kraw", bufs=4) as kraw, \
             tc.tile_pool(name="knt", bufs=3) as kntp, \
             tc.tile_pool(name="segout", bufs=1) as segp, \
             tc.tile_pool(name="tpsum", bufs=2, space="PSUM") as tpsum, \
             tc.tile_pool(name="mpsum", bufs=3, space="PSUM") as mpsum:

            qc = constp.tile([D, B + TP], bf16, tag="qc")
            nc.gpsimd.dma_start(out=qc[:], in_=qc_d[:])
            qnT = qc[:, :B]
            ident = qc[:, B:]

            seg = segp.tile([D, NB, G], f32, tag="seg")

            # fused per-chunk pipeline: DMA (R keys packed per partition,
            # 1KB contiguous bf16 runs) -> 4 PE transposes -> ACT copy ->
            # 2 matmuls -> 1 segmented reduce.  Column c of chunk k holds
            # key k*W + R*(c % TP) + c // TP (host undoes the packing).
            dma_engines = [nc.sync]
            for k in range(NCH):
                kt = kraw.tile([TP, R * D], bf16, tag="kt")
                dma_engines[k % len(dma_engines)].dma_start(
                    out=kt[:],
                    in_=keys_d[k * W:(k + 1) * W, :].rearrange(
                        "(p j) d -> p (j d)", j=R),
                )
                ps = tpsum.tile([D, 512], bf16, tag="tp")
                for j in range(NT):
                    nc.tensor.transpose(
                        out=ps[:, j * TP:(j + 1) * TP],
                        in_=kt[:, j * D:(j + 1) * D],
                        identity=ident,
                    )
                knt = kntp.tile([D, W], bf16, tag="knt")
                if k == 0:
                    # DVE is idle during the ramp and skips the ACT
                    # activation-table load on the first chunk's chain.
                    nc.vector.tensor_copy(knt[:], ps[:])
                else:
                    nc.scalar.copy(out=knt[:], in_=ps[:])

                mp = mpsum.tile([D, NB, 512], f32, tag="mp")
                for pb in range(NB):
                    nc.tensor.matmul(
                        out=mp[:, pb, :],
                        lhsT=qnT[:, pb * 128:(pb + 1) * 128],
                        rhs=knt[:],
                        start=True, stop=True,
                    )
                red_in = mp[:].rearrange("p a (g s) -> p a g s", s=SEG)
                nc.vector.reduce_max(
                    out=seg[:, :, k * SPC:(k + 1) * SPC],
                    in_=red_in,
                    axis=mybir.AxisListType.X,
                )
                if k == SPLIT_K:
                    # drain the finished half of the output early so the
                    # kernel tail only pays for the remainder.
                    g0 = (SPLIT_K + 1) * SPC
                    for pb in range(NB):
                        nc.sync.dma_start(
                            out=segmax_d[pb * 128:(pb + 1) * 128, :g0],
                            in_=seg[:, pb, :g0],
                        )

            g0 = (SPLIT_K + 1) * SPC
            for pb in range(NB):
                nc.sync.dma_start(
                    out=segmax_d[pb * 128:(pb + 1) * 128, g0:],
                    in_=seg[:, pb, g0:],
                )

    nc.finalize()
    return nc


def _get_nc():
    if "nc" not in _NC_CACHE:
        _NC_CACHE["nc"] = _build_bass()
    return _NC_CACHE["nc"]


def _install_trace_hook():
    """Make trace=True work under axon: provide the antenv.axon_hooks
    shim the boot script expects and register the ctypes NTFF hook."""
    import types

    try:
        from antenv.axon_hooks import get_axon_ntff_profile_hook  # noqa: F401
        return True
    except ImportError:
        pass
    try:
        import antenv
        from trn_agent_boot.trn_boot import _ntff_profile_via_ctypes

        m = types.ModuleType("antenv.axon_hooks")
        _state = {"hook": None}
        m.set_axon_ntff_profile_hook = lambda h: _state.__setitem__("hook", h)
        m.get_axon_ntff_profile_hook = lambda: _state["hook"]
        sys.modules["antenv.axon_hooks"] = m
        antenv.axon_hooks = m
        hook = _ntff_profile_via_ctypes("/opt/axon/libaxon_pjrt.so")
        if hook is None:
            return False
        m.set_axon_ntff_profile_hook(hook)

        # artifact upload needs bucket creds the container lacks; keep
        # the profile local instead.
        from concourse import bass_utils as bu
        bu.upload_artifacts = lambda tmpdir: f"local://{tmpdir}"
        return True
    except Exception as e:  # pragma: no cover
        print("trace hook install failed:", e)
        return False


def _host_topk(A, qn, keys_scaled, top_k, M):
    """Select top-k elements per query given device seg-maxima A [B, NSEG]."""
    nseg = A.shape[1]
    M = min(M, nseg)
    idx = np.argpartition(-A, M - 1, axis=1)[:, :M]           # [B, M] seg ids
    if M < nseg:
        thr = -np.partition(-A, M, axis=1)[:, M]              # (M+1)-th seg max
    else:
        thr = np.full(A.shape[0], -np.inf, dtype=A.dtype)
    cols = (idx[:, :, None] * SEG
            + np.arange(SEG, dtype=np.int64)).reshape(A.shape[0], M * SEG)
    cand = np.sort(COL2KEY[cols], axis=1)     # -1 pad entries sort first
    valid = cand >= 0
    kc = keys_scaled[np.where(valid, cand, 0)]                 # [B, M*SEG, D]
    sims = np.matmul(kc, qn[:, :, None], dtype=np.float32)[:, :, 0]
    sims[~valid] = -np.inf
    order = np.argsort(-sims, axis=1, kind="stable")[:, :top_k]
    top_idx = np.take_along_axis(cand, order, axis=1)
    top_sims = np.take_along_axis(sims, order, axis=1)
    safe = top_sims[:, -1] >= thr + MARGIN
    return top_idx, top_sims, safe


def kernel(**inputs):
    global LAST_EXEC_TIME_NS, LAST_RESULTS

    query = np.ascontiguousarray(np.asarray(inputs["query"], dtype=np.float32))
    keys = np.ascontiguousarray(np.asarray(inputs["keys"], dtype=np.float32))
    values = np.asarray(inputs["values"], dtype=np.float32)
    ts = np.asarray(inputs["timestamps"])
    gs = int(np.asarray(inputs["global_step"]))
    top_k = int(np.asarray(inputs["top_k"]))

    # --- host prep -------------------------------------------------------
    qn = query / np.maximum(
        np.linalg.norm(query, axis=-1, keepdims=True), NORM_EPS)
    qn = qn.astype(np.float32)
    qnT = np.ascontiguousarray(qn.T)

    age = (np.int64(gs) - ts.astype(np.int64)).astype(np.float32)
    decay = np.power(np.float32(DECAY_FACTOR), age).astype(np.float32)
    knorm = np.sqrt(np.einsum("cd,cd->c", keys, keys)).astype(np.float32)
    scale = decay / np.maximum(knorm, NORM_EPS)
    keys_scaled = keys * scale[:, None]

    import ml_dtypes
    qc = np.zeros((D, B + TP), dtype=ml_dtypes.bfloat16)
    qc[:, :B] = qnT.astype(ml_dtypes.bfloat16)
    qc[:, B:] = np.eye(D, dtype=ml_dtypes.bfloat16)
    keys_bf = np.zeros((N_CORES * CD, D), dtype=ml_dtypes.bfloat16)
    keys_bf.reshape(N_CORES, CD, D)[:, :C, :] = \
        keys_scaled.astype(ml_dtypes.bfloat16).reshape(N_CORES, C, D)
    in_maps = [
        {"qc": qc, "keys": keys_bf[i * CD:(i + 1) * CD]}
        for i in range(N_CORES)
    ]

    # --- device ----------------------------------------------------------
    from concourse.bass_utils import run_bass_kernel_spmd

    nc = _get_nc()
    trace = os.environ.get("BASS_KERNEL_TRACE", "0") == "1"
    kw = {}
    if trace and _install_trace_hook():
        kw = {"trace": True,
              "trace_cores": list(range(N_CORES))}
    try:
        res = run_bass_kernel_spmd(nc, in_maps, list(range(N_CORES)), **kw)
    except Exception:
        if not kw:
            raise
        res = run_bass_kernel_spmd(nc, in_maps, list(range(N_CORES)))
    LAST_EXEC_TIME_NS = res.exec_time_ns
    LAST_RESULTS = res

    A = np.concatenate(
        [res.results[i]["segmax"] for i in range(N_CORES)], axis=1)

    # --- host reduce ------------------------------------------------------
    M = max(12, top_k)
    while True:
        top_idx, top_sims, safe = _host_topk(A, qn, keys_scaled, top_k, M)
        if safe.all() or M >= A.shape[1]:
            break
        M = min(M * 4, A.shape[1])

    retrieved = values[top_idx]
    valid_mask = top_sims >= np.float32(MIN_SIMILARITY)
    return (retrieved.astype(np.float32),
            top_sims.astype(np.float32),
            valid_mask,
            top_idx.astype(np.int32))


# revision 46
# speedup vs baseline: 1.0669x; 1.0193x over previous
"""ErrorMemoryBank.retrieve kernel for 8 TRN2 NeuronCores.

Strategy (bank sharded on the capacity axis, 12500 keys per core):
  host prep : fold temporal decay and key L2-norm into a per-key scale,
              pre-scale the keys once (shipped as bf16), L2-normalize +
              transpose the query.
  device    : per core, stream the key shard through the PE array
              (transpose + bf16 matmul against the 128x256 query blocks)
              and reduce the 256x12800 similarity matrix to per-segment
              maxima (segments of 32 keys) on the vector engine.  Only
              [256, 400] f32 seg-maxima leave each core.
  host post : pick the top-M segments per query from the 8*400 global
              seg-maxima, recompute exact fp32 similarities for those few
              candidates, take the global top-k (lowest-index
              tie-breaking, matching jax.lax.top_k), gather values.

The union of the top-k segments (ranked by segment max) always contains
the top-k elements, so M >= top_k plus a generous margin + widening
fallback makes host selection exact up to fp rounding.
"""

import os
import sys

import numpy as np

for _p in ("/opt/trn_rl_repo",):
    if _p not in sys.path:
        sys.path.insert(0, _p)

N_CORES = 8
B, D = 256, 128
CAP = 100000
C = CAP // N_CORES   # 12500 real keys per core
W = 512              # matmul chunk width (columns)
R = 4                # keys packed per SBUF partition (1KB runs)
TP = W // R          # 128 partitions per raw keys tile
NT = R               # transpose strips per chunk
NCH = 25             # chunks per core
CD = NCH * W         # 12800 device keys per core (padded with zeros)
SEG = 32             # keys per segment (seg-max granularity)
SPC = W // SEG       # 16 segments per chunk
G = NCH * SPC        # 400 segments per core
NB = B // 128        # 2 query partition blocks
SPLIT_K = 19         # chunk after which the first output half is drained

DECAY_FACTOR = 0.995
MIN_SIMILARITY = 0.0
NORM_EPS = 1e-12
MARGIN = 1e-2        # device (bf16) seg-max error allowance for the fallback

LAST_EXEC_TIME_NS = None
LAST_RESULTS = None

_NC_CACHE = {}

# The per-chunk DMA packs R consecutive keys into each SBUF partition
# (2KB contiguous runs), so sims column c of chunk k corresponds to
# per-core key k*W + R*(c % TP) + c // TP.  Each core is padded from C
# to CD keys; COL2KEY maps a global sims column to its global key id,
# with -1 marking pad columns.
_gc = np.arange(N_CORES * CD, dtype=np.int64)
_core = _gc // CD
_lc = _gc % CD
_cc = _lc % W
_lkey = (_lc - _cc) + R * (_cc % TP) + _cc // TP
COL2KEY = np.where(_lkey < C, _core * C + _lkey, -1)
del _gc, _core, _lc, _cc, _lkey


def _build_bass():
    import concourse.mybir as mybir
    from concourse import bacc
    from concourse.tile import TileContext

    f32 = mybir.dt.float32
    bf16 = mybir.dt.bfloat16

    nc = bacc.Bacc(trn_type="TRN2")
    # qc packs the transposed query [D, B] and a DxD identity (for the PE
    # transposes) into one input so a single early DMA covers both.
    qc_d = nc.declare_dram_parameter("qc", [D, B + TP], bf16, isOutput=False)
    keys_d = nc.declare_dram_parameter("keys", [CD, D], bf16, isOutput=False)
    segmax_d = nc.declare_dram_parameter("segmax", [B, G], f32, isOutput=True)

    with TileContext(nc) as tc:
        with tc.tile_pool(name="const", bufs=1) as constp, \
             tc.tile_pool(name="kraw", bufs=4) as kraw, \
             tc.tile_pool(name="knt", bufs=3) as kntp, \
             tc.tile_pool(name="segout", bufs=1) as segp, \
             tc.tile_pool(name="tpsum", bufs=2, space="PSUM") as tpsum, \
             tc.tile_pool(name="mpsum", bufs=3, space="PSUM") as mpsum:

            qc = constp.tile([D, B + TP], bf16, tag="qc")
            nc.gpsimd.dma_start(out=qc[:], in_=qc_d[:])
            qnT = qc[:, :B]
            ident = qc[:, B:]

            seg = segp.tile([D, NB, G], f32, tag="seg")

            # fused per-chunk pipeline: DMA (R keys packed per partition,
            # 1KB contiguous bf16 runs) -> 4 PE transposes -> ACT copy ->
            # 2 matmuls -> 1 segmented reduce.  Column c of chunk k holds
            # key k*W + R*(c % TP) + c // TP (host undoes the packing).
            dma_engines = [nc.sync]
            for k in range(NCH):
                kt = kraw.tile([TP, R * D], bf16, tag="kt")
                dma_engines[k % len(dma_engines)].dma_start(
                    out=kt[:],
                    in_=keys_d[k * W:(k + 1) * W, :].rearrange(
                        "(p j) d -> p (j d)", j=R),
                )
                ps = tpsum.tile([D, 512], f32, tag="tp")
                for j in range(NT):
                    # transpose as a REGULAR matmul (kt_strip.T @ I): the
                    # HAM clock-gate ignores transpose-mode, so plain
                    # matmuls keep the PE at full clock between sims
                    # matmuls (no cold-throttle hiccups).
                    nc.tensor.matmul(
                        out=ps[:, j * TP:(j + 1) * TP],
                        lhsT=kt[:, j * D:(j + 1) * D],
                        rhs=ident,
                        start=True, stop=True,
                    )
                knt = kntp.tile([D, W], bf16, tag="knt")
                if k == 0:
                    # DVE is idle during the ramp and skips the ACT
                    # activation-table load on the first chunk's chain.
                    nc.vector.tensor_copy(knt[:], ps[:])
                else:
                    nc.scalar.copy(out=knt[:], in_=ps[:])

                mp = mpsum.tile([D, NB, 512], f32, tag="mp")
                for pb in range(NB):
                    nc.tensor.matmul(
                        out=mp[:, pb, :],
                        lhsT=qnT[:, pb * 128:(pb + 1) * 128],
                        rhs=knt[:],
                        start=True, stop=True,
                    )
                red_in = mp[:].rearrange("p a (g s) -> p a g s", s=SEG)
                nc.vector.reduce_max(
                    out=seg[:, :, k * SPC:(k + 1) * SPC],
                    in_=red_in,
                    axis=mybir.AxisListType.X,
                )
                if k == SPLIT_K:
                    # drain the finished half of the output early so the
                    # kernel tail only pays for the remainder.
                    g0 = (SPLIT_K + 1) * SPC
                    for pb in range(NB):
                        nc.sync.dma_start(
                            out=segmax_d[pb * 128:(pb + 1) * 128, :g0],
                            in_=seg[:, pb, :g0],
                        )

            g0 = (SPLIT_K + 1) * SPC
            for pb in range(NB):
                nc.sync.dma_start(
                    out=segmax_d[pb * 128:(pb + 1) * 128, g0:],
                    in_=seg[:, pb, g0:],
                )

    nc.finalize()
    return nc


def _get_nc():
    if "nc" not in _NC_CACHE:
        _NC_CACHE["nc"] = _build_bass()
    return _NC_CACHE["nc"]


def _install_trace_hook():
    """Make trace=True work under axon: provide the antenv.axon_hooks
    shim the boot script expects and register the ctypes NTFF hook."""
    import types

    try:
        from antenv.axon_hooks import get_axon_ntff_profile_hook  # noqa: F401
        return True
    except ImportError:
        pass
    try:
        import antenv
        from trn_agent_boot.trn_boot import _ntff_profile_via_ctypes

        m = types.ModuleType("antenv.axon_hooks")
        _state = {"hook": None}
        m.set_axon_ntff_profile_hook = lambda h: _state.__setitem__("hook", h)
        m.get_axon_ntff_profile_hook = lambda: _state["hook"]
        sys.modules["antenv.axon_hooks"] = m
        antenv.axon_hooks = m
        hook = _ntff_profile_via_ctypes("/opt/axon/libaxon_pjrt.so")
        if hook is None:
            return False
        m.set_axon_ntff_profile_hook(hook)

        # artifact upload needs bucket creds the container lacks; keep
        # the profile local instead.
        from concourse import bass_utils as bu
        bu.upload_artifacts = lambda tmpdir: f"local://{tmpdir}"
        return True
    except Exception as e:  # pragma: no cover
        print("trace hook install failed:", e)
        return False


def _host_topk(A, qn, keys_scaled, top_k, M):
    """Select top-k elements per query given device seg-maxima A [B, NSEG]."""
    nseg = A.shape[1]
    M = min(M, nseg)
    idx = np.argpartition(-A, M - 1, axis=1)[:, :M]           # [B, M] seg ids
    if M < nseg:
        thr = -np.partition(-A, M, axis=1)[:, M]              # (M+1)-th seg max
    else:
        thr = np.full(A.shape[0], -np.inf, dtype=A.dtype)
    cols = (idx[:, :, None] * SEG
            + np.arange(SEG, dtype=np.int64)).reshape(A.shape[0], M * SEG)
    cand = np.sort(COL2KEY[cols], axis=1)     # -1 pad entries sort first
    valid = cand >= 0
    kc = keys_scaled[np.where(valid, cand, 0)]                 # [B, M*SEG, D]
    sims = np.matmul(kc, qn[:, :, None], dtype=np.float32)[:, :, 0]
    sims[~valid] = -np.inf
    order = np.argsort(-sims, axis=1, kind="stable")[:, :top_k]
    top_idx = np.take_along_axis(cand, order, axis=1)
    top_sims = np.take_along_axis(sims, order, axis=1)
    safe = top_sims[:, -1] >= thr + MARGIN
    return top_idx, top_sims, safe


def kernel(**inputs):
    global LAST_EXEC_TIME_NS, LAST_RESULTS

    query = np.ascontiguousarray(np.asarray(inputs["query"], dtype=np.float32))
    keys = np.ascontiguousarray(np.asarray(inputs["keys"], dtype=np.float32))
    values = np.asarray(inputs["values"], dtype=np.float32)
    ts = np.asarray(inputs["timestamps"])
    gs = int(np.asarray(inputs["global_step"]))
    top_k = int(np.asarray(inputs["top_k"]))

    # --- host prep -------------------------------------------------------
    qn = query / np.maximum(
        np.linalg.norm(query, axis=-1, keepdims=True), NORM_EPS)
    qn = qn.astype(np.float32)
    qnT = np.ascontiguousarray(qn.T)

    age = (np.int64(gs) - ts.astype(np.int64)).astype(np.float32)
    decay = np.power(np.float32(DECAY_FACTOR), age).astype(np.float32)
    knorm = np.sqrt(np.einsum("cd,cd->c", keys, keys)).astype(np.float32)
    scale = decay / np.maximum(knorm, NORM_EPS)
    keys_scaled = keys * scale[:, None]

    import ml_dtypes
    qc = np.zeros((D, B + TP), dtype=ml_dtypes.bfloat16)
    qc[:, :B] = qnT.astype(ml_dtypes.bfloat16)
    qc[:, B:] = np.eye(D, dtype=ml_dtypes.bfloat16)
    keys_bf = np.zeros((N_CORES * CD, D), dtype=ml_dtypes.bfloat16)
    keys_bf.reshape(N_CORES, CD, D)[:, :C, :] = \
        keys_scaled.astype(ml_dtypes.bfloat16).reshape(N_CORES, C, D)
    in_maps = [
        {"qc": qc, "keys": keys_bf[i * CD:(i + 1) * CD]}
        for i in range(N_CORES)
    ]

    # --- device ----------------------------------------------------------
    from concourse.bass_utils import run_bass_kernel_spmd

    nc = _get_nc()
    trace = os.environ.get("BASS_KERNEL_TRACE", "0") == "1"
    kw = {}
    if trace and _install_trace_hook():
        kw = {"trace": True,
              "trace_cores": list(range(N_CORES))}
    try:
        res = run_bass_kernel_spmd(nc, in_maps, list(range(N_CORES)), **kw)
    except Exception:
        if not kw:
            raise
        res = run_bass_kernel_spmd(nc, in_maps, list(range(N_CORES)))
    LAST_EXEC_TIME_NS = res.exec_time_ns
    LAST_RESULTS = res

    A = np.concatenate(
        [res.results[i]["segmax"] for i in range(N_CORES)], axis=1)

    # --- host reduce ------------------------------------------------------
    M = max(12, top_k)
    while True:
        top_idx, top_sims, safe = _host_topk(A, qn, keys_scaled, top_k, M)
        if safe.all() or M >= A.shape[1]:
            break
        M = min(M * 4, A.shape[1])

    retrieved = values[top_idx]
    valid_mask = top_sims >= np.float32(MIN_SIMILARITY)
    return (retrieved.astype(np.float32),
            top_sims.astype(np.float32),
            valid_mask,
            top_idx.astype(np.int32))
